# revision 37
# baseline (speedup 1.0000x reference)
"""AttentionBlock (GroupNorm + 1x1-conv QKV + full NxN attention + proj +
residual) on 8 Trainium2 NeuronCores, data-parallel over the batch dim.

Per core: 2 samples of x[16, 512, 32, 32]. Matmul operands are fp8e4m3
with DoubleRow perf mode (2 contraction planes per instruction — the
128x128 PE array virtualizes to 128x256, halving the matmul count vs
bf16; fp32 matmul would be 4 cycles/row). PSUM accumulation and the
x + ... residual path stay fp32, so the output error stays ~1e-3
relative (the residual dominates the output).

Algebraic folds done on the host (exact):
  - GroupNorm affine (norm_w, norm_b) folded into qkv_w / qkv_b.
  - V bias and proj bias folded into one per-channel bias
    pb' = proj_w @ bv + proj_b  (softmax rows sum to 1).
  - softmax denominator folded through the projection:
    out = (x + pb') + (proj_w @ (V @ E^T)) * (1/Z).

On-chip dataflow per sample (C=512, N=1024 pixels):
  x [C,N] fp32 (c on partitions, 4 p-tiles) -> GroupNorm stats via
  bn_stats (last tile via ACT accumulate) + two tiny one-hot matmuls
  (partition-dim reduce / broadcast; rstd = exp(-0.5*ln(var+eps)) keeps
  ACT on one table set) -> xn fp8 -> Q,K [c,n] and V^T [n,c] via
  matmuls with host-pre-transposed weights -> S^T[j,i] = sum_c K Q
  (both i-blocks' S phases emitted back-to-back so the ACT exps never
  gate the PE) -> E = exp(S^T/sqrt(C)) fp8 (no max subtraction:
  |S|/sqrt(C) <= ~0.5 for unit-normal inputs) -> Z row-sums via
  ones-matmul, 1/Z = exp(-ln(Z)) on ACT, broadcast across partitions
  with a K=1 matmul -> O = (V @ E^T) * (1/Z) folded into the PSUM
  evacuation -> Y = proj(O) -> out = (x + pb') + Y.
"""

import math
import sys

import numpy as np

try:
    import concourse.bass as bass
except ImportError:  # pragma: no cover - grading container path setup
    sys.path.insert(0, "/opt/trn_rl_repo")
    import concourse.bass as bass

import bass_rust
import ml_dtypes
import concourse.tile as tile
from concourse import mybir
from concourse.bass_utils import run_bass_kernel_spmd

F32 = mybir.dt.float32
BF16 = mybir.dt.bfloat16
FP8 = mybir.dt.float8e4
DR = mybir.MatmulPerfMode.DoubleRow
AF = mybir.ActivationFunctionType
OP = mybir.AluOpType

NCORES = 8
B = 16
S = B // NCORES  # samples per core
C = 512
N = 1024  # H*W
G = 8  # groups
EPS = 1e-5
CT = C // 128  # channel p-tiles (4)
NT = N // 128  # spatial p-tiles (8)
IBS = 512  # i-block size
IB = N // IBS  # i blocks (2)
INV_SQRT_C = 1.0 / math.sqrt(C)

# Settable by test harness for profiling; not used by the grader.
TRACE = False
LAST_RESULT = None


MAX_WAITS = 1


def _split_excess_waits(nc, max_waits=MAX_WAITS):
    """Workaround for a walrus codegen limit: an instruction may carry at
    most `max_waits` semaphore waits ("Too many sync wait commands").
    Move the excess onto a chain of NOPs on the same engine right before
    the instruction — sequentially blocking waits on one engine queue are
    semantically identical to one multi-wait instruction."""
    counter = 0
    for f in nc.m.functions:
        for blk in f.blocks:
            il = blk.instructions
            if not any(
                i.sync_info is not None and len(i.sync_info.on_wait) > max_waits
                for i in il
            ):
                continue
            old = list(il)
            il.clear()
            for ins in old:
                si = ins.sync_info
                waits = list(si.on_wait) if si is not None else []
                if len(waits) > max_waits:
                    excess, keep = waits[:-max_waits], waits[-max_waits:]
                    for i0 in range(0, len(excess), max_waits):
                        counter += 1
                        nop = mybir.InstNoOp(
                            name=f"waitsplit-{counter}",
                            engine=ins.engine,
                            ins=[],
                            outs=[],
                            sync_info=bass_rust.SyncInfo(
                                on_wait=excess[i0 : i0 + max_waits], on_update=[]
                            ),
                        )
                        nc.register_instruction(nop, overwrite=True)
                        blk.add_instruction(nop)
                    ins.sync_info = bass_rust.SyncInfo(
                        on_wait=keep, on_update=list(si.on_update)
                    )
                blk.add_instruction(ins)
    return counter


def _build():
    from contextlib import ExitStack

    nc = bass.Bass()
    xd = nc.declare_dram_parameter("x", [S, C, N], F32, isOutput=False)
    wald = nc.declare_dram_parameter("wall", [128, 4 * CT, C], FP8, isOutput=False)
    bald = nc.declare_dram_parameter("ball", [128, 3 * CT], F32, isOutput=False)
    cad = nc.declare_dram_parameter("ca", [128, 2], BF16, isOutput=False)
    cbd = nc.declare_dram_parameter("cb", [2, 128], BF16, isOutput=False)
    outd = nc.declare_dram_parameter("out", [S, C, N], F32, isOutput=True)

    x_ap = xd[:].rearrange("s (t p) n -> s t p n", p=128)
    out_ap = outd[:].rearrange("s (t p) n -> s p t n", p=128)

    with tile.TileContext(nc) as tc, ExitStack() as ctx:
        singles = ctx.enter_context(tc.tile_pool(name="singles", bufs=1))
        xp = ctx.enter_context(tc.tile_pool(name="xp", bufs=S))
        xnp = ctx.enter_context(tc.tile_pool(name="xnp", bufs=S))
        qp = ctx.enter_context(tc.tile_pool(name="qp", bufs=2))
        kp = ctx.enter_context(tc.tile_pool(name="kp", bufs=2))
        vp = ctx.enter_context(tc.tile_pool(name="vp", bufs=2))
        ep = ctx.enter_context(tc.tile_pool(name="ep", bufs=12))
        op_ = ctx.enter_context(tc.tile_pool(name="op", bufs=3))
        rzp = ctx.enter_context(tc.tile_pool(name="rzp", bufs=2))
        obp = ctx.enter_context(tc.tile_pool(name="obp", bufs=4))
        smp = ctx.enter_context(tc.tile_pool(name="smp", bufs=4))
        ps = ctx.enter_context(tc.tile_pool(name="ps", bufs=1, space="PSUM"))

        # ---- input DMAs: x first (GroupNorm is the critical path) ----
        xs = []
        for s in range(S):
            x_sb = xp.tile([128, CT, N], F32, tag="x")
            xs.append(x_sb)
            for t in range(CT):
                nc.sync.dma_start(out=x_sb[:, t, :], in_=x_ap[s, t])
            if s == 0:
                wall = singles.tile([128, 4 * CT, C], FP8)
                nc.sync.dma_start(out=wall[:], in_=wald[:])
                ball = singles.tile([128, 3 * CT], F32)
                nc.gpsimd.dma_start(out=ball[:], in_=bald[:])
                ca = singles.tile([128, 2], BF16)
                nc.gpsimd.dma_start(out=ca[:], in_=cad[:])
                cb = singles.tile([2, 128], BF16)
                nc.gpsimd.dma_start(out=cb[:], in_=cbd[:])
                eps_sb = singles.tile([2, 1], F32)
                nc.vector.memset(eps_sb[:], EPS)
                ones8 = singles.tile([128, 2, 16], FP8)
                nc.vector.memset(ones8[:], 1.0)
                ones_row = singles.tile([1, 128], BF16)
                nc.vector.memset(ones_row[:], 1.0)
        # weight planes for DoubleRow: [p, wi, g, q, o]; channel = 256g+128q+p
        w8 = wall.rearrange("p (w g q) f -> p w g q f", g=2, q=2)
        bq, bk, pb = (ball[:, i * CT : (i + 1) * CT] for i in range(3))
        ghot, hhot = ca[:, 0:2], cb[:, :]

        xns = [None] * S

        def emit_gn(s):
            x_sb = xs[s]
            xn_g = [
                xnp.tile([128, 2, N], FP8, tag="xn", bufs=2 * S, name=f"xn{s}{g}")
                for g in range(2)
            ]
            xns[s] = xn_g
            sa_h = [
                smp.tile([128, 4], BF16, tag="sa", name=f"sa{s}{h}")
                for h in range(2)
            ]

            def dve_stats(t):
                sa = sa_h[t // 2]
                st6 = smp.tile([128, 2, 6], F32, tag="st6")
                nc.vector.bn_stats(out=st6[:, 0, :], in_=x_sb[:, t, 0:512])
                nc.vector.bn_stats(out=st6[:, 1, :], in_=x_sb[:, t, 512:1024])
                mv = smp.tile([128, 2], F32, tag="mv")
                nc.vector.bn_aggr(out=mv[:], in_=st6[:])
                nc.vector.tensor_copy(
                    out=sa[:, 2 * (t % 2) : 2 * (t % 2) + 1], in_=mv[:, 0:1]
                )
                msq = smp.tile([128, 1], F32, tag="msq")
                nc.vector.tensor_mul(msq[:], mv[:, 0:1], mv[:, 0:1])
                nc.vector.tensor_tensor(
                    out=sa[:, 2 * (t % 2) + 1 : 2 * (t % 2) + 2],
                    in0=mv[:, 1:2],
                    in1=msq[:],
                    op=OP.add,
                )

            def act_stats(t):
                # stats on ACT (otherwise idle): sum / sum-of-squares
                sa = sa_h[t // 2]
                scr = smp.tile([128, N], BF16, tag="scr")
                sum3 = smp.tile([128, 1], F32, tag="sum3")
                nc.scalar.activation(
                    out=scr[:], in_=x_sb[:, t, :], func=AF.Identity,
                    accum_out=sum3[:],
                )
                scr2 = smp.tile([128, N], BF16, tag="scr")
                sq3 = smp.tile([128, 1], F32, tag="sq3")
                nc.scalar.activation(
                    out=scr2[:], in_=x_sb[:, t, :], func=AF.Square,
                    accum_out=sq3[:],
                )
                c0 = 2 * (t % 2)
                nc.scalar.activation(
                    out=sa[:, c0 : c0 + 1], in_=sum3[:], func=AF.Identity,
                    scale=1.0 / N,
                )
                nc.scalar.activation(
                    out=sa[:, c0 + 1 : c0 + 2], in_=sq3[:], func=AF.Identity,
                    scale=1.0 / N,
                )

            def chain(h0):
                # group reduce -> rstd -> broadcast -> apply for tile pair h0
                gs_ps = ps.tile([2, 4], F32, tag="gn", bufs=1)
                nc.tensor.matmul(
                    gs_ps[:], lhsT=ghot, rhs=sa_h[h0][:], start=True, stop=True
                )
                gs3 = gs_ps.rearrange("h (t s) -> h t s", s=2)
                sq = smp.tile([2, 2], F32, tag="sq")
                nc.scalar.activation(out=sq[:], in_=gs3[:, :, 0], func=AF.Square)
                var = smp.tile([2, 2], F32, tag="var")
                nc.vector.tensor_tensor(
                    out=var[:], in0=gs3[:, :, 1], in1=sq[:], op=OP.subtract
                )
                lnv = smp.tile([2, 2], F32, tag="lnv")
                nc.scalar.activation(
                    out=lnv[:], in_=var[:], func=AF.Ln, bias=eps_sb[:], scale=1.0
                )
                # vals: (rstd, mean*rstd); apply is x*rstd - mean*rstd
                vals = smp.tile([2, 4], BF16, tag="vals")
                vals3 = vals.rearrange("h (t s) -> h t s", s=2)
                nc.scalar.activation(
                    out=vals3[:, :, 0], in_=lnv[:], func=AF.Exp, scale=-0.5
                )
                nc.vector.tensor_tensor(
                    out=vals3[:, :, 1], in0=gs3[:, :, 0], in1=vals3[:, :, 0],
                    op=OP.mult,
                )
                bc = ps.tile([128, 4], F32, tag="gn", bufs=1)
                nc.tensor.matmul(
                    bc[:], lhsT=hhot, rhs=vals[:], start=True, stop=True
                )
                bcs = smp.tile([128, 4], F32, tag="bcs")
                nc.vector.tensor_copy(out=bcs[:], in_=bc[:])
                for tt in range(2):
                    t = 2 * h0 + tt
                    nc.vector.tensor_scalar(
                        out=xn_g[h0][:, tt, :],
                        in0=x_sb[:, t, :],
                        scalar1=bcs[:, 2 * tt : 2 * tt + 1],
                        scalar2=bcs[:, 2 * tt + 1 : 2 * tt + 2],
                        op0=OP.mult,
                        op1=OP.subtract,
                    )

            # emission order = engine queue order: all stats ahead of the
            # chains on their engines so the last apply (which gates QKV's
            # full-contraction matmuls) lands as early as possible; the
            # x += pb' residual-base updates go last (not on any path)
            act_stats(0)
            dve_stats(1)
            dve_stats(2)
            dve_stats(3)
            chain(0)
            chain(1)
            for t in range(CT):
                nc.vector.tensor_scalar(
                    out=x_sb[:, t, :],
                    in0=x_sb[:, t, :],
                    scalar1=pb[:, t : t + 1],
                    scalar2=None,
                    op0=OP.add,
                )

        qkvs = [None] * S

        def emit_qkv(s):
            xn_g = xns[s]
            q_sb = qp.tile([128, 2, 2, N], FP8, tag="q")
            k_sb = kp.tile([128, 2, 2, N], FP8, tag="k")
            v_sb = vp.tile([128, NT // 2, 2, C], FP8, tag="v")
            qkvs[s] = (q_sb, k_sb, v_sb)
            qv = q_sb.rearrange("p g q n -> p (g q) n")
            kv = k_sb.rearrange("p g q n -> p (g q) n")
            vv = v_sb.rearrange("p g q n -> p (g q) n")
            for wi, b_sb, dst in ((0, bq, qv), (1, bk, kv)):
                for ot in range(CT):
                    for ib in range(IB):
                        psm = ps.tile([128, IBS], F32, tag="mm", bufs=5)
                        for g in range(2):
                            nc.tensor.matmul(
                                psm[:],
                                lhsT=w8[:, wi, g, :, ot * 128 : (ot + 1) * 128],
                                rhs=xn_g[g][:, :, ib * IBS : (ib + 1) * IBS],
                                start=(g == 0),
                                stop=(g == 1),
                                perf_mode=DR,
                            )
                        if wi == 0:
                            nc.scalar.activation(
                                out=dst[:, ot, ib * IBS : (ib + 1) * IBS],
                                in_=psm[:],
                                func=AF.Identity,
                                bias=b_sb[:, ot : ot + 1],
                                scale=1.0,
                            )
                        else:
                            nc.vector.tensor_scalar(
                                out=dst[:, ot, ib * IBS : (ib + 1) * IBS],
                                in0=psm[:],
                                scalar1=b_sb[:, ot : ot + 1],
                                scalar2=None,
                                op0=OP.add,
                            )
            for nt in range(NT):
                psm = ps.tile([128, IBS], F32, tag="mm", bufs=5)
                for g in range(2):
                    nc.tensor.matmul(
                        psm[:],
                        lhsT=xn_g[g][:, :, nt * 128 : (nt + 1) * 128],
                        rhs=w8[:, 2, g, :, :],
                        start=(g == 0),
                        stop=(g == 1),
                        perf_mode=DR,
                    )
                nc.vector.tensor_copy(out=vv[:, nt, :], in_=psm[:])

        def emit_attn(s):
            x_sb = xs[s]
            q_sb, k_sb, v_sb = qkvs[s]
            es_ib = []
            for ib in range(IB):
                isl = slice(ib * IBS, (ib + 1) * IBS)
                es = []
                es_ib.append(es)
                for jt in range(NT):
                    psm = ps.tile([128, IBS], F32, tag="mm", bufs=5)
                    for g in range(2):
                        nc.tensor.matmul(
                            psm[:],
                            lhsT=k_sb[:, g, :, jt * 128 : (jt + 1) * 128],
                            rhs=q_sb[:, g, :, isl],
                            start=(g == 0),
                            stop=(g == 1),
                            perf_mode=DR,
                        )
                    if jt % 2 == 0:
                        e = ep.tile([128, 2, IBS], FP8, tag="e")
                        es.append(e)
                    nc.scalar.activation(
                        out=es[jt // 2][:, jt % 2, :],
                        in_=psm[:],
                        func=AF.Exp,
                        scale=INV_SQRT_C,
                    )
            for ib in range(IB):
                isl = slice(ib * IBS, (ib + 1) * IBS)
                es = es_ib[ib]
                zps = ps.tile([1, IBS], F32, tag="z", bufs=1)
                for jg in range(NT // 2):
                    nc.tensor.matmul(
                        zps[:],
                        lhsT=ones8[:, :, 0:1],
                        rhs=es[jg][:],
                        start=(jg == 0),
                        stop=(jg == NT // 2 - 1),
                        perf_mode=DR,
                    )
                # 1/Z = exp(-ln(Z)) on ACT (DVE reciprocal on a 1-partition
                # row costs ~3.3us and stalls the evacuations)
                lnz = rzp.tile([1, IBS], F32, tag="lnz")
                nc.scalar.activation(out=lnz[:], in_=zps[:], func=AF.Ln)
                rz = rzp.tile([1, IBS], BF16, tag="rz")
                nc.scalar.activation(out=rz[:], in_=lnz[:], func=AF.Exp, scale=-1.0)
                o_sb = op_.tile([128, 2, 2, IBS], FP8, tag="o")
                ov = o_sb.rearrange("p g q n -> p (g q) n")
                zb = obp.tile([128, IBS], F32, tag="zbs", bufs=2)
                for ct in range(CT):
                    psm = ps.tile([128, IBS], F32, tag="mm", bufs=5)
                    for jg in range(NT // 2):
                        nc.tensor.matmul(
                            psm[:],
                            lhsT=v_sb[:, jg, :, ct * 128 : (ct + 1) * 128],
                            rhs=es[jg][:],
                            start=(jg == 0),
                            stop=(jg == NT // 2 - 1),
                            perf_mode=DR,
                        )
                    if ct == 0:
                        zb_ps = ps.tile([128, IBS], F32, tag="zb", bufs=1)
                        nc.tensor.matmul(
                            zb_ps[:], lhsT=ones_row, rhs=rz[:], start=True, stop=True
                        )
                        nc.vector.tensor_copy(out=zb[:], in_=zb_ps[:])
                    # fold softmax normalization into the evacuation
                    nc.vector.tensor_tensor(
                        out=ov[:, ct, :], in0=psm[:], in1=zb[:], op=OP.mult
                    )
                ob4 = obp.tile([128, CT, IBS], F32, tag="ob", bufs=3)
                for ot in range(CT):
                    psm = ps.tile([128, IBS], F32, tag="mm", bufs=5)
                    for g in range(2):
                        nc.tensor.matmul(
                            psm[:],
                            lhsT=w8[:, 3, g, :, ot * 128 : (ot + 1) * 128],
                            rhs=o_sb[:, g, :, :],
                            start=(g == 0),
                            stop=(g == 1),
                            perf_mode=DR,
                        )
                    nc.vector.tensor_add(
                        out=ob4[:, ot, :], in0=psm[:], in1=x_sb[:, ot, isl]
                    )
                    if s == S - 1 and ib == IB - 1:
                        nc.sync.dma_start(
                            out=out_ap[s][:, ot : ot + 1, isl],
                            in_=ob4[:, ot : ot + 1, :],
                        )
                if not (s == S - 1 and ib == IB - 1):
                    nc.sync.dma_start(out=out_ap[s][:, :, isl], in_=ob4[:])

        emit_gn(0)
        emit_qkv(0)
        emit_gn(1)
        emit_attn(0)
        emit_qkv(1)
        emit_attn(1)

    _split_excess_waits(nc)
    return nc


_NC = None


def kernel(x, norm_w, norm_b, qkv_w, qkv_b, proj_w, proj_b):
    global _NC, LAST_RESULT
    x = np.ascontiguousarray(np.asarray(x, dtype=np.float32))
    norm_w = np.asarray(norm_w, dtype=np.float32)
    norm_b = np.asarray(norm_b, dtype=np.float32)
    qkv_w = np.asarray(qkv_w, dtype=np.float32)
    qkv_b = np.asarray(qkv_b, dtype=np.float32)
    proj_w = np.asarray(proj_w, dtype=np.float32)
    proj_b = np.asarray(proj_b, dtype=np.float32)

    # fold GroupNorm affine into qkv
    wq_full = qkv_w * norm_w[None, :]
    bq_full = qkv_b + qkv_w @ norm_b
    wq_, wk_, wv_ = wq_full[0:C], wq_full[C : 2 * C], wq_full[2 * C : 3 * C]
    bq_, bk_, bv_ = bq_full[0:C], bq_full[C : 2 * C], bq_full[2 * C : 3 * C]
    pb_ = proj_w @ bv_ + proj_b

    def wtile(w):  # [o, c] -> DoubleRow lhsT planes [128, 2(g), 2(q), o]
        return w.T.reshape(2, 2, 128, C).transpose(2, 0, 1, 3)

    def btile(b):  # [C] -> [128, ct]
        return b.reshape(CT, 128).T

    wall = np.ascontiguousarray(
        np.stack(
            [wtile(wq_), wtile(wk_), wtile(wv_), wtile(proj_w)], axis=1
        ).reshape(128, 16, C).astype(ml_dtypes.float8_e4m3)
    )
    ball = np.ascontiguousarray(
        np.concatenate([btile(bq_), btile(bk_), btile(pb_)], axis=1).astype(
            np.float32
        )
    )
    cl = np.arange(128)
    ghot = np.zeros((128, 2), np.float32)
    ghot[cl, cl // 64] = 1.0 / 64.0
    hhot = np.zeros((2, 128), np.float32)
    hhot[cl // 64, cl] = 1.0

    common = {
        "wall": wall,
        "ball": ball,
        "ca": ghot.astype(ml_dtypes.bfloat16),
        "cb": hhot.astype(ml_dtypes.bfloat16),
    }
    xr = x.reshape(NCORES, S, C, N)
    in_maps = [dict(common, x=np.ascontiguousarray(xr[i])) for i in range(NCORES)]

    if _NC is None:
        _NC = _build()
    res = run_bass_kernel_spmd(
        _NC, in_maps, core_ids=list(range(NCORES)), trace=TRACE
    )
    LAST_RESULT = res
    out = np.stack([res.results[i]["out"] for i in range(NCORES)])
    return np.ascontiguousarray(out.reshape(B, C, 32, 32).astype(np.float32))


# revision 38
# speedup vs baseline: 1.1562x; 1.1562x over previous
"""AttentionBlock (GroupNorm + 1x1-conv QKV + full NxN attention + proj +
residual) on 8 Trainium2 NeuronCores, data-parallel over the batch dim.

Per core: 2 samples of x[16, 512, 32, 32]. Matmul operands are fp8e4m3
with DoubleRow perf mode (2 contraction planes per instruction — the
128x128 PE array virtualizes to 128x256, halving the matmul count vs
bf16; fp32 matmul would be 4 cycles/row). PSUM accumulation and the
x + ... residual path stay fp32, so the output error stays ~1e-3
relative (the residual dominates the output).

Algebraic folds done on the host (exact):
  - GroupNorm affine (norm_w, norm_b) folded into qkv_w / qkv_b.
  - V bias and proj bias folded into one per-channel bias
    pb' = proj_w @ bv + proj_b  (softmax rows sum to 1).
  - softmax denominator folded through the projection:
    out = (x + pb') + (proj_w @ (V @ E^T)) * (1/Z).

On-chip dataflow per sample (C=512, N=1024 pixels):
  x [C,N] fp32 (c on partitions, 4 p-tiles) -> GroupNorm stats via
  bn_stats (last tile via ACT accumulate) + two tiny one-hot matmuls
  (partition-dim reduce / broadcast; rstd = exp(-0.5*ln(var+eps)) keeps
  ACT on one table set) -> xn fp8 -> Q,K [c,n] and V^T [n,c] via
  matmuls with host-pre-transposed weights -> S^T[j,i] = sum_c K Q
  (both i-blocks' S phases emitted back-to-back so the ACT exps never
  gate the PE) -> E = exp(S^T/sqrt(C)) fp8 (no max subtraction:
  |S|/sqrt(C) <= ~0.5 for unit-normal inputs) -> Z row-sums via
  ones-matmul, 1/Z = exp(-ln(Z)) on ACT, broadcast across partitions
  with a K=1 matmul -> O = (V @ E^T) * (1/Z) folded into the PSUM
  evacuation -> Y = proj(O) -> out = (x + pb') + Y.
"""

import math
import sys

import numpy as np

try:
    import concourse.bass as bass
except ImportError:  # pragma: no cover - grading container path setup
    sys.path.insert(0, "/opt/trn_rl_repo")
    import concourse.bass as bass

import bass_rust
import ml_dtypes
import concourse.tile as tile
from concourse import mybir
from concourse.bass_utils import run_bass_kernel_spmd

F32 = mybir.dt.float32
BF16 = mybir.dt.bfloat16
FP8 = mybir.dt.float8e4
DR = mybir.MatmulPerfMode.DoubleRow
AF = mybir.ActivationFunctionType
OP = mybir.AluOpType

NCORES = 8
B = 16
S = B // NCORES  # samples per core
C = 512
N = 1024  # H*W
G = 8  # groups
EPS = 1e-5
CT = C // 128  # channel p-tiles (4)
NT = N // 128  # spatial p-tiles (8)
IBS = 512  # i-block size
IB = N // IBS  # i blocks (2)
INV_SQRT_C = 1.0 / math.sqrt(C)

# Settable by test harness for profiling; not used by the grader.
TRACE = False
LAST_RESULT = None


MAX_WAITS = 1


def _split_excess_waits(nc, max_waits=MAX_WAITS):
    """Workaround for a walrus codegen limit: an instruction may carry at
    most `max_waits` semaphore waits ("Too many sync wait commands").
    Move the excess onto a chain of NOPs on the same engine right before
    the instruction — sequentially blocking waits on one engine queue are
    semantically identical to one multi-wait instruction."""
    counter = 0
    for f in nc.m.functions:
        for blk in f.blocks:
            il = blk.instructions
            if not any(
                i.sync_info is not None and len(i.sync_info.on_wait) > max_waits
                for i in il
            ):
                continue
            old = list(il)
            il.clear()
            for ins in old:
                si = ins.sync_info
                waits = list(si.on_wait) if si is not None else []
                if len(waits) > max_waits:
                    excess, keep = waits[:-max_waits], waits[-max_waits:]
                    for i0 in range(0, len(excess), max_waits):
                        counter += 1
                        nop = mybir.InstNoOp(
                            name=f"waitsplit-{counter}",
                            engine=ins.engine,
                            ins=[],
                            outs=[],
                            sync_info=bass_rust.SyncInfo(
                                on_wait=excess[i0 : i0 + max_waits], on_update=[]
                            ),
                        )
                        nc.register_instruction(nop, overwrite=True)
                        blk.add_instruction(nop)
                    ins.sync_info = bass_rust.SyncInfo(
                        on_wait=keep, on_update=list(si.on_update)
                    )
                blk.add_instruction(ins)
    return counter


def _build():
    from contextlib import ExitStack

    nc = bass.Bass()
    xd = nc.declare_dram_parameter("x", [S, C, N], F32, isOutput=False)
    wald = nc.declare_dram_parameter("wall", [128, 4 * CT, C], FP8, isOutput=False)
    bald = nc.declare_dram_parameter("ball", [128, 3 * CT], F32, isOutput=False)
    cad = nc.declare_dram_parameter("ca", [128, 2], BF16, isOutput=False)
    cbd = nc.declare_dram_parameter("cb", [2, 128], BF16, isOutput=False)
    outd = nc.declare_dram_parameter("out", [S, C, N], F32, isOutput=True)

    x_ap = xd[:].rearrange("s (t p) n -> s t p n", p=128)
    out_ap = outd[:].rearrange("s (t p) n -> s p t n", p=128)

    with tile.TileContext(nc) as tc, ExitStack() as ctx:
        singles = ctx.enter_context(tc.tile_pool(name="singles", bufs=1))
        xp = ctx.enter_context(tc.tile_pool(name="xp", bufs=S))
        xnp = ctx.enter_context(tc.tile_pool(name="xnp", bufs=S))
        qp = ctx.enter_context(tc.tile_pool(name="qp", bufs=2))
        kp = ctx.enter_context(tc.tile_pool(name="kp", bufs=2))
        vp = ctx.enter_context(tc.tile_pool(name="vp", bufs=2))
        ep = ctx.enter_context(tc.tile_pool(name="ep", bufs=16))
        op_ = ctx.enter_context(tc.tile_pool(name="op", bufs=4))
        rzp = ctx.enter_context(tc.tile_pool(name="rzp", bufs=2))
        obp = ctx.enter_context(tc.tile_pool(name="obp", bufs=6))
        smp = ctx.enter_context(tc.tile_pool(name="smp", bufs=4))
        ps = ctx.enter_context(tc.tile_pool(name="ps", bufs=1, space="PSUM"))

        # ---- input DMAs: x first (GroupNorm is the critical path) ----
        xs = []
        for s in range(S):
            x_sb = xp.tile([128, CT, N], F32, tag="x")
            xs.append(x_sb)
            for t in range(CT):
                nc.sync.dma_start(out=x_sb[:, t, :], in_=x_ap[s, t])
            if s == 0:
                wall = singles.tile([128, 4 * CT, C], FP8)
                nc.sync.dma_start(out=wall[:], in_=wald[:])
                ball = singles.tile([128, 3 * CT], F32)
                nc.gpsimd.dma_start(out=ball[:], in_=bald[:])
                ca = singles.tile([128, 2], BF16)
                nc.gpsimd.dma_start(out=ca[:], in_=cad[:])
                cb = singles.tile([2, 128], BF16)
                nc.gpsimd.dma_start(out=cb[:], in_=cbd[:])
                eps_sb = singles.tile([2, 1], F32)
                nc.vector.memset(eps_sb[:], EPS)
                ones8 = singles.tile([128, 2, 16], FP8)
                nc.vector.memset(ones8[:], 1.0)
                ones_row = singles.tile([1, 128], BF16)
                nc.vector.memset(ones_row[:], 1.0)
        # weight planes for DoubleRow: [p, wi, g, q, o]; channel = 256g+128q+p
        w8 = wall.rearrange("p (w g q) f -> p w g q f", g=2, q=2)
        bq, bk, pb = (ball[:, i * CT : (i + 1) * CT] for i in range(3))
        ghot, hhot = ca[:, 0:2], cb[:, :]

        xns = [None] * S

        def emit_gn(s):
            x_sb = xs[s]
            xn_g = [
                xnp.tile([128, 2, N], FP8, tag="xn", bufs=2 * S, name=f"xn{s}{g}")
                for g in range(2)
            ]
            xns[s] = xn_g
            sa_h = [
                smp.tile([128, 4], BF16, tag="sa", name=f"sa{s}{h}")
                for h in range(2)
            ]

            def dve_stats(t):
                sa = sa_h[t // 2]
                st6 = smp.tile([128, 2, 6], F32, tag="st6")
                nc.vector.bn_stats(out=st6[:, 0, :], in_=x_sb[:, t, 0:512])
                nc.vector.bn_stats(out=st6[:, 1, :], in_=x_sb[:, t, 512:1024])
                mv = smp.tile([128, 2], F32, tag="mv")
                nc.vector.bn_aggr(out=mv[:], in_=st6[:])
                nc.vector.tensor_copy(
                    out=sa[:, 2 * (t % 2) : 2 * (t % 2) + 1], in_=mv[:, 0:1]
                )
                msq = smp.tile([128, 1], F32, tag="msq")
                nc.vector.tensor_mul(msq[:], mv[:, 0:1], mv[:, 0:1])
                nc.vector.tensor_tensor(
                    out=sa[:, 2 * (t % 2) + 1 : 2 * (t % 2) + 2],
                    in0=mv[:, 1:2],
                    in1=msq[:],
                    op=OP.add,
                )

            def act_stats(t):
                # stats on ACT (otherwise idle): sum / sum-of-squares
                sa = sa_h[t // 2]
                scr = smp.tile([128, N], BF16, tag="scr")
                sum3 = smp.tile([128, 1], F32, tag="sum3")
                nc.scalar.activation(
                    out=scr[:], in_=x_sb[:, t, :], func=AF.Identity,
                    accum_out=sum3[:],
                )
                scr2 = smp.tile([128, N], BF16, tag="scr")
                sq3 = smp.tile([128, 1], F32, tag="sq3")
                nc.scalar.activation(
                    out=scr2[:], in_=x_sb[:, t, :], func=AF.Square,
                    accum_out=sq3[:],
                )
                c0 = 2 * (t % 2)
                nc.scalar.activation(
                    out=sa[:, c0 : c0 + 1], in_=sum3[:], func=AF.Identity,
                    scale=1.0 / N,
                )
                nc.scalar.activation(
                    out=sa[:, c0 + 1 : c0 + 2], in_=sq3[:], func=AF.Identity,
                    scale=1.0 / N,
                )

            def chain(h0):
                # group reduce -> rstd -> broadcast -> apply for tile pair h0
                gs_ps = ps.tile([2, 4], F32, tag="gn", bufs=1)
                nc.tensor.matmul(
                    gs_ps[:], lhsT=ghot, rhs=sa_h[h0][:], start=True, stop=True
                )
                gs3 = gs_ps.rearrange("h (t s) -> h t s", s=2)
                sq = smp.tile([2, 2], F32, tag="sq")
                nc.scalar.activation(out=sq[:], in_=gs3[:, :, 0], func=AF.Square)
                var = smp.tile([2, 2], F32, tag="var")
                nc.vector.tensor_tensor(
                    out=var[:], in0=gs3[:, :, 1], in1=sq[:], op=OP.subtract
                )
                lnv = smp.tile([2, 2], F32, tag="lnv")
                nc.scalar.activation(
                    out=lnv[:], in_=var[:], func=AF.Ln, bias=eps_sb[:], scale=1.0
                )
                # vals: (rstd, mean*rstd); apply is x*rstd - mean*rstd
                vals = smp.tile([2, 4], BF16, tag="vals")
                vals3 = vals.rearrange("h (t s) -> h t s", s=2)
                nc.scalar.activation(
                    out=vals3[:, :, 0], in_=lnv[:], func=AF.Exp, scale=-0.5
                )
                nc.vector.tensor_tensor(
                    out=vals3[:, :, 1], in0=gs3[:, :, 0], in1=vals3[:, :, 0],
                    op=OP.mult,
                )
                bc = ps.tile([128, 4], F32, tag="gn", bufs=1)
                nc.tensor.matmul(
                    bc[:], lhsT=hhot, rhs=vals[:], start=True, stop=True
                )
                bcs = smp.tile([128, 4], F32, tag="bcs")
                nc.vector.tensor_copy(out=bcs[:], in_=bc[:])
                for tt in range(2):
                    t = 2 * h0 + tt
                    nc.vector.tensor_scalar(
                        out=xn_g[h0][:, tt, :],
                        in0=x_sb[:, t, :],
                        scalar1=bcs[:, 2 * tt : 2 * tt + 1],
                        scalar2=bcs[:, 2 * tt + 1 : 2 * tt + 2],
                        op0=OP.mult,
                        op1=OP.subtract,
                    )

            # emission order = engine queue order: all stats ahead of the
            # chains on their engines so the last apply (which gates QKV's
            # full-contraction matmuls) lands as early as possible; the
            # x += pb' residual-base updates go last (not on any path)
            act_stats(0)
            dve_stats(1)
            dve_stats(2)
            dve_stats(3)
            chain(0)
            chain(1)
            for t in range(CT):
                nc.vector.tensor_scalar(
                    out=x_sb[:, t, :],
                    in0=x_sb[:, t, :],
                    scalar1=pb[:, t : t + 1],
                    scalar2=None,
                    op0=OP.add,
                )

        qkvs = [None] * S

        def emit_qkv(s):
            xn_g = xns[s]
            q_sb = qp.tile([128, 2, 2, N], FP8, tag="q")
            k_sb = kp.tile([128, 2, 2, N], FP8, tag="k")
            v_sb = vp.tile([128, NT // 2, 2, C], FP8, tag="v")
            qkvs[s] = (q_sb, k_sb, v_sb)
            qv = q_sb.rearrange("p g q n -> p (g q) n")
            kv = k_sb.rearrange("p g q n -> p (g q) n")
            vv = v_sb.rearrange("p g q n -> p (g q) n")
            for wi, b_sb, dst in ((0, bq, qv), (1, bk, kv)):
                for ot in range(CT):
                    for ib in range(IB):
                        psm = ps.tile([128, IBS], F32, tag="mm", bufs=5)
                        for g in range(2):
                            nc.tensor.matmul(
                                psm[:],
                                lhsT=w8[:, wi, g, :, ot * 128 : (ot + 1) * 128],
                                rhs=xn_g[g][:, :, ib * IBS : (ib + 1) * IBS],
                                start=(g == 0),
                                stop=(g == 1),
                                perf_mode=DR,
                            )
                        if wi == 0:
                            nc.scalar.activation(
                                out=dst[:, ot, ib * IBS : (ib + 1) * IBS],
                                in_=psm[:],
                                func=AF.Identity,
                                bias=b_sb[:, ot : ot + 1],
                                scale=1.0,
                            )
                        else:
                            nc.vector.tensor_scalar(
                                out=dst[:, ot, ib * IBS : (ib + 1) * IBS],
                                in0=psm[:],
                                scalar1=b_sb[:, ot : ot + 1],
                                scalar2=None,
                                op0=OP.add,
                            )
            for nt in range(NT):
                psm = ps.tile([128, IBS], F32, tag="mm", bufs=5)
                for g in range(2):
                    nc.tensor.matmul(
                        psm[:],
                        lhsT=xn_g[g][:, :, nt * 128 : (nt + 1) * 128],
                        rhs=w8[:, 2, g, :, :],
                        start=(g == 0),
                        stop=(g == 1),
                        perf_mode=DR,
                    )
                nc.vector.tensor_copy(out=vv[:, nt, :], in_=psm[:])

        def emit_attn(s):
            x_sb = xs[s]
            q_sb, k_sb, v_sb = qkvs[s]
            es_ib = []
            for ib in range(IB):
                isl = slice(ib * IBS, (ib + 1) * IBS)
                es = []
                es_ib.append(es)
                for jt in range(NT):
                    psm = ps.tile([128, IBS], F32, tag="mm", bufs=5)
                    for g in range(2):
                        nc.tensor.matmul(
                            psm[:],
                            lhsT=k_sb[:, g, :, jt * 128 : (jt + 1) * 128],
                            rhs=q_sb[:, g, :, isl],
                            start=(g == 0),
                            stop=(g == 1),
                            perf_mode=DR,
                        )
                    if jt % 2 == 0:
                        e = ep.tile([128, 2, IBS], FP8, tag="e")
                        es.append(e)
                    nc.scalar.activation(
                        out=es[jt // 2][:, jt % 2, :],
                        in_=psm[:],
                        func=AF.Exp,
                        scale=INV_SQRT_C,
                    )
            for ib in range(IB):
                isl = slice(ib * IBS, (ib + 1) * IBS)
                es = es_ib[ib]
                zps = ps.tile([1, IBS], F32, tag="z", bufs=1)
                for jg in range(NT // 2):
                    nc.tensor.matmul(
                        zps[:],
                        lhsT=ones8[:, :, 0:1],
                        rhs=es[jg][:],
                        start=(jg == 0),
                        stop=(jg == NT // 2 - 1),
                        perf_mode=DR,
                    )
                # 1/Z = exp(-ln(Z)) on ACT (DVE reciprocal on a 1-partition
                # row costs ~3.3us and stalls the evacuations)
                lnz = rzp.tile([1, IBS], F32, tag="lnz")
                nc.scalar.activation(out=lnz[:], in_=zps[:], func=AF.Ln)
                rz = rzp.tile([1, IBS], BF16, tag="rz")
                nc.scalar.activation(out=rz[:], in_=lnz[:], func=AF.Exp, scale=-1.0)
                o_sb = op_.tile([128, 2, 2, IBS], FP8, tag="o")
                ov = o_sb.rearrange("p g q n -> p (g q) n")
                zb = obp.tile([128, IBS], F32, tag="zbs", bufs=2)
                for ct in range(CT):
                    psm = ps.tile([128, IBS], F32, tag="mm", bufs=5)
                    for jg in range(NT // 2):
                        nc.tensor.matmul(
                            psm[:],
                            lhsT=v_sb[:, jg, :, ct * 128 : (ct + 1) * 128],
                            rhs=es[jg][:],
                            start=(jg == 0),
                            stop=(jg == NT // 2 - 1),
                            perf_mode=DR,
                        )
                    if ct == 0:
                        zb_ps = ps.tile([128, IBS], F32, tag="zb", bufs=1)
                        nc.tensor.matmul(
                            zb_ps[:], lhsT=ones_row, rhs=rz[:], start=True, stop=True
                        )
                        nc.vector.tensor_copy(out=zb[:], in_=zb_ps[:])
                    # fold softmax normalization into the evacuation
                    nc.vector.tensor_tensor(
                        out=ov[:, ct, :], in0=psm[:], in1=zb[:], op=OP.mult
                    )
                ob4 = obp.tile([128, CT, IBS], F32, tag="ob", bufs=3)
                for ot in range(CT):
                    psm = ps.tile([128, IBS], F32, tag="mm", bufs=5)
                    for g in range(2):
                        nc.tensor.matmul(
                            psm[:],
                            lhsT=w8[:, 3, g, :, ot * 128 : (ot + 1) * 128],
                            rhs=o_sb[:, g, :, :],
                            start=(g == 0),
                            stop=(g == 1),
                            perf_mode=DR,
                        )
                    nc.vector.tensor_add(
                        out=ob4[:, ot, :], in0=psm[:], in1=x_sb[:, ot, isl]
                    )
                    if s == S - 1 and ib == IB - 1:
                        nc.sync.dma_start(
                            out=out_ap[s][:, ot : ot + 1, isl],
                            in_=ob4[:, ot : ot + 1, :],
                        )
                if not (s == S - 1 and ib == IB - 1):
                    nc.sync.dma_start(out=out_ap[s][:, :, isl], in_=ob4[:])

        emit_gn(0)
        emit_qkv(0)
        emit_gn(1)
        emit_attn(0)
        emit_qkv(1)
        emit_attn(1)

    _split_excess_waits(nc)
    return nc


_NC = None


def kernel(x, norm_w, norm_b, qkv_w, qkv_b, proj_w, proj_b):
    global _NC, LAST_RESULT
    x = np.ascontiguousarray(np.asarray(x, dtype=np.float32))
    norm_w = np.asarray(norm_w, dtype=np.float32)
    norm_b = np.asarray(norm_b, dtype=np.float32)
    qkv_w = np.asarray(qkv_w, dtype=np.float32)
    qkv_b = np.asarray(qkv_b, dtype=np.float32)
    proj_w = np.asarray(proj_w, dtype=np.float32)
    proj_b = np.asarray(proj_b, dtype=np.float32)

    # fold GroupNorm affine into qkv
    wq_full = qkv_w * norm_w[None, :]
    bq_full = qkv_b + qkv_w @ norm_b
    wq_, wk_, wv_ = wq_full[0:C], wq_full[C : 2 * C], wq_full[2 * C : 3 * C]
    bq_, bk_, bv_ = bq_full[0:C], bq_full[C : 2 * C], bq_full[2 * C : 3 * C]
    pb_ = proj_w @ bv_ + proj_b

    def wtile(w):  # [o, c] -> DoubleRow lhsT planes [128, 2(g), 2(q), o]
        return w.T.reshape(2, 2, 128, C).transpose(2, 0, 1, 3)

    def btile(b):  # [C] -> [128, ct]
        return b.reshape(CT, 128).T

    wall = np.ascontiguousarray(
        np.stack(
            [wtile(wq_), wtile(wk_), wtile(wv_), wtile(proj_w)], axis=1
        ).reshape(128, 16, C).astype(ml_dtypes.float8_e4m3)
    )
    ball = np.ascontiguousarray(
        np.concatenate([btile(bq_), btile(bk_), btile(pb_)], axis=1).astype(
            np.float32
        )
    )
    cl = np.arange(128)
    ghot = np.zeros((128, 2), np.float32)
    ghot[cl, cl // 64] = 1.0 / 64.0
    hhot = np.zeros((2, 128), np.float32)
    hhot[cl // 64, cl] = 1.0

    common = {
        "wall": wall,
        "ball": ball,
        "ca": ghot.astype(ml_dtypes.bfloat16),
        "cb": hhot.astype(ml_dtypes.bfloat16),
    }
    xr = x.reshape(NCORES, S, C, N)
    in_maps = [dict(common, x=np.ascontiguousarray(xr[i])) for i in range(NCORES)]

    if _NC is None:
        _NC = _build()
    res = run_bass_kernel_spmd(
        _NC, in_maps, core_ids=list(range(NCORES)), trace=TRACE
    )
    LAST_RESULT = res
    out = np.stack([res.results[i]["out"] for i in range(NCORES)])
    return np.ascontiguousarray(out.reshape(B, C, 32, 32).astype(np.float32))
